# revision 53
# baseline (speedup 1.0000x reference)
"""Distributed Trainium2 (Bass/Tile) kernel for the GNN+MLP model.

Strategy (8 NeuronCores, SPMD):
  - Node tables (d_ecfps / p_gos) row-sharded across cores; each core computes
    h = x @ W for its rows, scales by dinv (degree^-1/2) -> local g table.
  - GCN aggregation is batch-expanded (one output row per batch slot) and
    source-sharded: each core accumulates the partial sums its local g rows
    contribute, expressed as small matmuls   psum += M_chunk^T-free @ G_chunk
    where M_chunk[e, slot] = edge_weight (built on-device from edge weights
    via an iota/compare trick) and G_chunk = dma_gather of local g rows.
  - Partials are ReduceScattered (fp16) so each core ends with its own 512
    batch rows, fully reduced; epilogue applies dinv[dest], bias, leaky.
  - The MLP (encoder/decoder/output head) runs batch-sharded per core in
    feature-major layout (weights as natural [in,out] matmul operands).

All host-side work is pure data marshalling: sharding, padding, index
layout, replication.  No arithmetic on float inputs happens on the host.
"""

import numpy as np

import concourse.bass as bass
import concourse.bacc as bacc
import concourse.mybir as mybir
import concourse.tile as tile
from concourse import library_config
from concourse.bass_utils import run_bass_kernel_spmd

dt = mybir.dt
F32, F16, I16 = dt.float32, dt.float16, dt.int16
AX = mybir.AxisListType
ALU = mybir.AluOpType
ACTF = mybir.ActivationFunctionType

NCORES = 8
P = 128


class Cfg:
    def __init__(self, nd=20000, np_=10000, b=4096, fd=1024, fp=2812, f=1024,
                 dvp=1324, enc_h=1024, enc_o=256, out_h=128):
        self.nd, self.np_, self.b = nd, np_, b
        self.fd = fd              # drug feature dim (in=out=1024)
        self.fp = fp              # protein in-feature dim (2812)
        self.f = f                # gcn out dim (1024)
        self.dvp = dvp            # d_vecs+p_embeddings cols (300+1024)
        self.enc_h, self.enc_o, self.out_h = enc_h, enc_o, out_h
        self.feat = dvp + 2 * f   # 3372
        assert nd % NCORES == 0 and np_ % NCORES == 0 and b % NCORES == 0
        self.nd_sh, self.np_sh = nd // NCORES, np_ // NCORES
        self.nd_pad = -(-self.nd_sh // P) * P
        self.np_pad = -(-self.np_sh // P) * P
        self.bsh = b // NCORES                 # batch rows per core
        assert self.bsh % P == 0
        self.nbt = self.bsh // P               # batch tiles per core (4)
        self.nbt_g = b // P                    # global batch tiles (32)


# ----------------------------------------------------------------------------
# host preprocessing (layout only)
# ----------------------------------------------------------------------------

def _csr(col, n):
    order = np.argsort(col, kind="stable")
    cs = col[order]
    indptr = np.searchsorted(cs, np.arange(n + 1))
    return order, indptr


def _graph_host(edge_index, edge_weight, idx_b, n, shard, pad, b):
    """Per-graph host structures: deg buckets + batch-expanded edge lists."""
    row = np.asarray(edge_index[0]).astype(np.int64)
    col = np.asarray(edge_index[1]).astype(np.int64)
    ew = np.asarray(edge_weight).astype(np.float32)
    j = np.asarray(idx_b).astype(np.int64)
    order, indptr = _csr(col, n)
    row_s, ew_s = row[order], ew[order]
    counts = np.diff(indptr)
    K = int(counts.max())
    # bucket[node, k] = k-th incoming edge weight (0-padded): deg = 1 + rowsum
    bucket = np.zeros((n, K), np.float32)
    pos = np.arange(len(col)) - np.repeat(indptr[:-1], counts)
    bucket[col[order], pos] = ew_s
    # batch-expanded edges: for each batch slot b0 with dest j[b0]
    cnt_b = counts[j]
    tot = int(cnt_b.sum())
    e_base = np.repeat(indptr[j], cnt_b)
    e_off = np.arange(tot) - np.repeat(np.cumsum(cnt_b) - cnt_b, cnt_b)
    e_idx = e_base + e_off
    src = row_s[e_idx]
    w = ew_s[e_idx]
    b_of = np.repeat(np.arange(b), cnt_b)
    # self loops (weight 1.0, src = dest)
    src = np.concatenate([src, j])
    w = np.concatenate([w, np.ones(b, np.float32)])
    b_of = np.concatenate([b_of, np.arange(b)])
    core = src // shard
    loc = (src - core * shard).astype(np.int64)
    return dict(bucket=bucket, K=K, core=core, loc=loc, w=w, b_of=b_of, j=j)


def _chunk_layout(g, cfg, nch=None):
    """Per-core gather/matmul chunk arrays for one graph.

    Returns idx_w [core][128, nbt_g*nch*8] i16 (dma_gather wrapped layout),
    ewT [core][128, nbt_g*nch] f32, dsT [core][128, nbt_g*nch] f16, nch.
    """
    nbt_g = cfg.nbt_g
    core, loc, w, b_of = g["core"], g["loc"], g["w"], g["b_of"]
    tile_id = b_of // P
    slot = b_of % P
    # max chunk count over (core, tile)
    cnt = np.zeros((NCORES, nbt_g), np.int64)
    np.add.at(cnt, (core, tile_id), 1)
    need = -(-int(cnt.max()) // P)
    if nch is None:
        nch = need
    assert nch >= need
    cap = nch * P
    idx_all, ew_all, ds_all = [], [], []
    key = core * nbt_g + tile_id
    order = np.argsort(key, kind="stable")
    ko, lo, wo, so = key[order], loc[order], w[order], slot[order]
    starts = np.searchsorted(ko, np.arange(NCORES * nbt_g))
    ends = np.searchsorted(ko, np.arange(NCORES * nbt_g) + 1)
    cnt_all = []
    for c in range(NCORES):
        idx = np.full((nbt_g, cap), -1, np.int16)
        eww = np.zeros((nbt_g, cap), np.float32)
        dss = np.zeros((nbt_g, cap), np.float32)
        cnts = np.zeros(nbt_g, np.int32)
        for t in range(nbt_g):
            s, e = starts[c * nbt_g + t], ends[c * nbt_g + t]
            m = e - s
            idx[t, :m] = lo[s:e]
            eww[t, :m] = wo[s:e]
            dss[t, :m] = so[s:e]
            if m == 0:
                idx[t, 0] = 0
                m = 1
            cnts[t] = m
        cnt_all.append(cnts)
        # wrap indices for dma_gather: index i -> (partition i%16, col i//16),
        # replicated 8x across partition groups; per tile block of cap//16 cols
        iw = idx.reshape(nbt_g, cap // 16, 16).transpose(2, 0, 1).reshape(16, -1)
        idx_all.append(np.tile(iw, (8, 1)))
        # chunk-transposed [128, nbt_g*nch]: entry (p, t*nch+ch) = edge ch*128+p
        ew_all.append(eww.reshape(nbt_g, nch, P).transpose(2, 0, 1).reshape(P, -1))
        ds_all.append(dss.reshape(nbt_g, nch, P).transpose(2, 0, 1).reshape(P, -1))
    return idx_all, ew_all, ds_all, nch, cnt_all


def _pad2(a, r, c):
    out = np.zeros((r, c), a.dtype)
    out[: a.shape[0], : a.shape[1]] = a
    return out


def host_plan(inputs, cfg):
    """Build per-core input maps.  Everything here is layout/sharding only."""
    ix = {k: np.asarray(v) for k, v in inputs.items()}
    c = cfg
    gd = _graph_host(ix["d_edge_index"], ix["d_edge_weight"], ix["d_index"],
                     c.nd, c.nd_sh, c.nd_pad, c.b)
    gp = _graph_host(ix["p_edge_index"], ix["p_edge_weight"], ix["p_index"],
                     c.np_, c.np_sh, c.np_pad, c.b)
    idx_d, ewT_d, dsT_d, nch_d, cnt_d = _chunk_layout(gd, c)
    idx_p, ewT_p, dsT_p, nch_p, cnt_p = _chunk_layout(gp, c)

    iota = np.broadcast_to(np.arange(P, dtype=np.float16), (P, P)).copy()
    ident = np.eye(P, dtype=np.float16)

    # reordered encoder W1 rows to match XT layout [ecfps | gos | dvp]
    W1 = ix["enc_W1"].astype(np.float32)
    W1r = np.concatenate([W1[c.dvp : c.dvp + c.f], W1[c.dvp + c.f :], W1[: c.dvp]], 0)

    def tile_bias(v, parts):
        # [n] -> [128, n//128] column tiles (per-partition layout)
        return np.asarray(v, np.float32).reshape(parts, P).T.copy()

    in_maps = []
    for cid in range(NCORES):
        b0 = cid * c.bsh
        bsl = slice(b0, b0 + c.bsh)
        m = {}
        # phase 1: transposed x shards (pad cols to *_pad), W replicated
        m["xdT"] = _pad2(ix["d_ecfps"][cid * c.nd_sh : (cid + 1) * c.nd_sh]
                         .astype(np.float32).T, c.fd, c.nd_pad)
        m["xpT"] = _pad2(ix["p_gos"][cid * c.np_sh : (cid + 1) * c.np_sh]
                         .astype(np.float32).T, -(-c.fp // P) * P, c.np_pad)
        m["Wd"] = ix["d_gcn_W"].astype(np.float32)
        m["Wp"] = _pad2(ix["p_gcn_W"].astype(np.float32), -(-c.fp // P) * P, c.f)
        m["bucket_d"] = _pad2(
            gd["bucket"][cid * c.nd_sh : (cid + 1) * c.nd_sh], c.nd_pad, gd["K"])
        m["bucket_p"] = _pad2(
            gp["bucket"][cid * c.np_sh : (cid + 1) * c.np_sh], c.np_pad, gp["K"])
        # batch-dest buckets (for dinv at destination), this core's batch rows
        m["bucket_bd"] = gd["bucket"][gd["j"][bsl]].astype(np.float32)
        m["bucket_bp"] = gp["bucket"][gp["j"][bsl]].astype(np.float32)
        # phase 2
        m["idx_d"], m["ewT_d"], m["dsT_d"] = idx_d[cid], ewT_d[cid], dsT_d[cid]
        m["idx_p"], m["ewT_p"], m["dsT_p"] = idx_p[cid], ewT_p[cid], dsT_p[cid]
        m["gcounts"] = np.stack([cnt_d[cid], cnt_p[cid]], 0)
        m["iota"], m["ident"] = iota, ident
        # phase 3
        dv = ix["d_vecs"][bsl].astype(np.float32)
        pe = ix["p_embeddings"][bsl].astype(np.float32)
        dvp_nat = np.concatenate([dv, pe], 1)          # [bsh, 1324]
        m["dvp_nat"] = dvp_nat
        m["dvpT"] = dvp_nat.T.copy()                   # [1324, bsh]
        m["gcn_bd"] = np.tile(ix["d_gcn_b"].astype(np.float32), (P, 1))
        m["gcn_bp"] = np.tile(ix["p_gcn_b"].astype(np.float32), (P, 1))
        m["W1r"] = W1r
        m["b1t"] = tile_bias(ix["enc_b1"], c.enc_h // P)
        m["W2"] = ix["enc_W2"].astype(np.float32)
        m["b2t"] = tile_bias(ix["enc_b2"], c.enc_o // P)
        m["D1"] = ix["dec_W1"].astype(np.float32)
        m["d1t"] = tile_bias(ix["dec_b1"], c.enc_h // P)
        m["D2"] = ix["dec_W2"].astype(np.float32)
        m["d2tile"] = np.tile(ix["dec_b2"].astype(np.float32), (P, 1))
        m["OW1"] = ix["out_W1"].astype(np.float32)
        m["ob1"] = ix["out_b1"].astype(np.float32).reshape(c.out_h, 1)
        m["bn_g"] = ix["bn_gamma"].astype(np.float32).reshape(c.out_h, 1)
        m["bn_b"] = ix["bn_beta"].astype(np.float32).reshape(c.out_h, 1)
        m["bn_m"] = ix["bn_mean"].astype(np.float32).reshape(c.out_h, 1)
        m["bn_v"] = ix["bn_var"].astype(np.float32).reshape(c.out_h, 1)
        m["OW2"] = ix["out_W2"].astype(np.float32)
        m["ob2"] = ix["out_b2"].astype(np.float32).reshape(1, 1)
        in_maps.append(m)
    meta = dict(K_d=gd["K"], K_p=gp["K"], nch_d=nch_d, nch_p=nch_p)
    return in_maps, meta


# ----------------------------------------------------------------------------
# device kernel
# ----------------------------------------------------------------------------

def build_kernel(nc, cfg, meta, phases=(1, 2, 25, 3), gather_runtime=True):
    c = cfg
    nch_d, nch_p = meta["nch_d"], meta["nch_p"]
    kch_d = c.fd // P
    kch_p = -(-c.fp // P)
    fh = c.f // 2                                  # 512 matmul N half

    def din(name, shape, dtype=F32):
        return nc.dram_tensor(name, list(shape), dtype, kind="ExternalInput").ap()

    def dout(name, shape):
        return nc.dram_tensor(name, list(shape), F32, kind="ExternalOutput").ap()

    # --- declare I/O ------------------------------------------------------
    xdT = din("xdT", (c.fd, c.nd_pad))
    xpT = din("xpT", (kch_p * P, c.np_pad))
    Wd = din("Wd", (c.fd, c.f))
    Wp = din("Wp", (kch_p * P, c.f))
    bucket_d = din("bucket_d", (c.nd_pad, meta["K_d"]))
    bucket_p = din("bucket_p", (c.np_pad, meta["K_p"]))
    bucket_bd = din("bucket_bd", (c.bsh, meta["K_d"]))
    bucket_bp = din("bucket_bp", (c.bsh, meta["K_p"]))
    idx_d = din("idx_d", (P, c.nbt_g * nch_d * 8), I16)
    idx_p = din("idx_p", (P, c.nbt_g * nch_p * 8), I16)
    ewT_d = din("ewT_d", (P, c.nbt_g * nch_d))
    ewT_p = din("ewT_p", (P, c.nbt_g * nch_p))
    dsT_d = din("dsT_d", (P, c.nbt_g * nch_d))
    dsT_p = din("dsT_p", (P, c.nbt_g * nch_p))
    gcounts = din("gcounts", (2, c.nbt_g), dt.int32)
    iota_in = din("iota", (P, P), F16)
    ident_in = din("ident", (P, P), F16)
    dvp_nat = din("dvp_nat", (c.bsh, c.dvp))
    dvpT = din("dvpT", (c.dvp, c.bsh))
    gcn_bd = din("gcn_bd", (P, c.f))
    gcn_bp = din("gcn_bp", (P, c.f))
    W1r = din("W1r", (c.feat, c.enc_h))
    b1t = din("b1t", (P, c.enc_h // P))
    W2 = din("W2", (c.enc_h, c.enc_o))
    b2t = din("b2t", (P, c.enc_o // P))
    D1 = din("D1", (c.enc_o, c.enc_h))
    d1t = din("d1t", (P, c.enc_h // P))
    D2 = din("D2", (c.enc_h, c.feat))
    d2tile = din("d2tile", (P, c.feat))
    OW1 = din("OW1", (c.enc_o, c.out_h))
    ob1 = din("ob1", (c.out_h, 1))
    bn_g = din("bn_g", (c.out_h, 1))
    bn_b = din("bn_b", (c.out_h, 1))
    bn_m = din("bn_m", (c.out_h, 1))
    bn_v = din("bn_v", (c.out_h, 1))
    OW2 = din("OW2", (c.out_h, 1))
    ob2 = din("ob2", (1, 1))

    y_o = dout("y_part", (c.bsh, 1))
    enc_o_t = dout("encoded_part", (c.bsh, c.enc_o))
    dec_o_t = dout("decoded_part", (c.bsh, c.feat))
    feat_o = dout("feature_part", (c.bsh, c.feat))

    nc.gpsimd.load_library(library_config.mlp)

    g_d = nc.dram_tensor("g_d", [c.nd_pad, c.f], F16, kind="Internal").ap()
    g_p = nc.dram_tensor("g_p", [c.np_pad, c.f], F16, kind="Internal").ap()
    part_in_d = nc.dram_tensor("part_in_d", [NCORES * c.bsh, c.f], F16,
                               kind="Internal").ap()
    _pa_rows = NCORES * (c.bsh // 2 if c.nbt >= 2 else c.bsh)
    part_in_pa = nc.dram_tensor("part_in_pa", [_pa_rows, c.f], F16,
                                kind="Internal").ap()
    part_in_pb = nc.dram_tensor("part_in_pb", [max(_pa_rows // 8, P), c.f]
                                if c.nbt < 2 else [_pa_rows, c.f], F16,
                                kind="Internal").ap()
    part_out_d = nc.dram_tensor("rs_out_d", [c.bsh, c.f], F16,
                                kind="Internal").ap()
    _po_rows = c.bsh // 2 if c.nbt >= 2 else c.bsh
    part_out_pa = nc.dram_tensor("rs_out_pa", [_po_rows, c.f], F16,
                                 kind="Internal").ap()
    part_out_pb = nc.dram_tensor("rs_out_pb", [max(_po_rows, P), c.f], F16,
                                 kind="Internal").ap()

    with tile.TileContext(nc) as tc:
        const = tc.alloc_tile_pool(name="const", bufs=1)
        cnt_sb = const.tile([1, 2 * c.nbt_g], dt.int32)
        nc.sync.dma_start(cnt_sb[:], gcounts.rearrange("a b -> (a b)")[None, :])
        iota_sb = const.tile([P, P], F16)
        nc.sync.dma_start(iota_sb[:], iota_in[:])
        ident_sb = const.tile([P, P], F16)
        nc.sync.dma_start(ident_sb[:], ident_in[:])

        mlp_p = tc.alloc_tile_pool(name="mlpw", bufs=1)
        # ---------------- phase 1: h = x @ W, g = dinv * h ----------------
        p1wr = tc.alloc_tile_pool(name="p1wr", bufs=1)

        def load_w(W, kch, tag):
            Wsb = p1wr.tile([P, kch, c.f], F16, tag=tag)
            wr = W.rearrange("(k p) n -> p k n", p=P)
            for k0 in range(0, kch, 4):
                k1 = min(k0 + 4, kch)
                nc.gpsimd.dma_start(Wsb[:, k0:k1, :], wr[:, k0:k1, :])
            return Wsb

        def calc_dinv(bucket, ntiles, tagsfx):
            dinv = const.tile([P, ntiles], F32, tag=f"dinv{tagsfx}")
            with tc.tile_pool(name="pdv", bufs=1) as pool:
                kb = bucket.shape[1]
                bt = pool.tile([P, ntiles, kb], F32, tag="bk")
                nc.sync.dma_start(
                    bt[:], bucket.rearrange("(t p) k -> p t k", p=P))
                ds = pool.tile([P, ntiles], F32, tag="ds")
                for t in range(ntiles):
                    nc.vector.reduce_sum(ds[:, t : t + 1], bt[:, t, :], axis=AX.X)
                dq = pool.tile([P, ntiles], F32, tag="dq")
                nc.scalar.activation(dq[:], ds[:], ACTF.Sqrt, bias=1.0)
                nc.vector.reciprocal(dinv[:], dq[:])
            return dinv

        def phase1(xT, Wsb, dinv, g_tab, ntiles, kch):
            with tc.tile_pool(name="p1", bufs=5) as pool, \
                 tc.tile_pool(name="p1ps", bufs=2, space="PSUM") as psp:
                for t in range(ntiles):
                    x16 = pool.tile([P, kch, P], F16, tag="x16")
                    nc.gpsimd.dma_start(
                        x16[:], xT.rearrange("(k p) n -> p k n", p=P)[
                            :, :, t * P : (t + 1) * P])
                    psa = psp.tile([P, fh], F32, tag="hpsa")
                    psb = psp.tile([P, fh], F32, tag="hpsb")
                    for k in range(kch):
                        for nh, pstile in enumerate((psa, psb)):
                            nc.tensor.matmul(
                                pstile[:],
                                lhsT=x16[:, k, :],
                                rhs=Wsb[:, k, nh * fh : (nh + 1) * fh],
                                start=(k == 0), stop=(k == kch - 1))
                    gt = pool.tile([P, c.f], F16, tag="gt")
                    nc.vector.tensor_scalar_mul(gt[:, :fh], psa[:], dinv[:, t : t + 1])
                    nc.vector.tensor_scalar_mul(gt[:, fh:], psb[:], dinv[:, t : t + 1])
                    nc.sync.dma_start(g_tab[t * P : (t + 1) * P, :], gt[:])
            return dinv

        if 1 in phases:
            Wd_sb = load_w(Wd, kch_d, "wd")
            dinv_d = calc_dinv(bucket_d, c.nd_pad // P, "d")
            phase1(xdT, Wd_sb, dinv_d, g_d, c.nd_pad // P, kch_d)

        # dinv at destinations for this core's batch rows: [128, nbt] per graph
        dinv_b = const.tile([P, 2 * c.nbt], F32)
        with tc.tile_pool(name="pb", bufs=2) as pool:
            for gi, bk in enumerate((bucket_bd, bucket_bp)):
                for t in range(c.nbt):
                    bt = pool.tile([P, bk.shape[1]], F32, tag="bkb")
                    nc.sync.dma_start(bt[:], bk[t * P : (t + 1) * P, :])
                    ds = pool.tile([P, 1], F32, tag="dsb")
                    nc.vector.reduce_sum(ds[:], bt[:], axis=AX.X)
                    dq = pool.tile([P, 1], F32, tag="dqb")
                    nc.scalar.activation(dq[:], ds[:], ACTF.Sqrt, bias=1.0)
                    nc.vector.reciprocal(dinv_b[:, gi * c.nbt + t : gi * c.nbt + t + 1], dq[:])

        # ---------------- MLP weight / XT preload (overlaps phase 2) -----
        kfeat = 2 * (c.f // P) + (c.dvp + P - 1) // P
        mh1 = c.enc_h // P
        m2 = c.enc_o // P
        XT = mlp_p.tile([P, kfeat, c.bsh], F16)
        W2sb = mlp_p.tile([P, mh1, c.enc_o], F16)
        D1sb = mlp_p.tile([P, m2, c.enc_h], F16)
        OW1sb = mlp_p.tile([P, m2, c.out_h], F16)
        d2b = mlp_p.tile([P, c.feat], F32)
        b1sb = mlp_p.tile([P, mh1], F32)
        b2sb = mlp_p.tile([P, m2], F32)
        d1sb = mlp_p.tile([P, mh1], F32)
        ow216 = mlp_p.tile([P, 1], F16)
        ob2sb = mlp_p.tile([1, 1], F32)
        vs = mlp_p.tile([P, 4], F32)
        sh = mlp_p.tile([P, 2], F32)
        gcn_b_sb = {}
        for gi, bsrc in enumerate((gcn_bd, gcn_bp)):
            bt = mlp_p.tile([P, c.f], F32, tag=f"gb{gi}")
            nc.sync.dma_start(bt[:], bsrc[:])
            gcn_b_sb[gi] = bt
        for k in range(mh1):
            nc.gpsimd.dma_start(W2sb[:, k, :], W2[k * P : (k + 1) * P, :])
        for k in range(m2):
            nc.gpsimd.dma_start(D1sb[:, k, :], D1[k * P : (k + 1) * P, :])
            nc.gpsimd.dma_start(OW1sb[:, k, :], OW1[k * P : (k + 1) * P, :])
        nc.sync.dma_start(d2b[:], d2tile[:])
        nc.sync.dma_start(b1sb[:], b1t[:])
        nc.sync.dma_start(b2sb[:], b2t[:])
        nc.sync.dma_start(d1sb[:], d1t[:])
        nc.gpsimd.dma_start(ow216[:], OW2[:])
        nc.sync.dma_start(ob2sb[:], ob2[:])
        with tc.tile_pool(name="pre3", bufs=2) as pool:
            small = mlp_p.tile([P, 8], F32)
            for i, src in enumerate((ob1, bn_g, bn_b, bn_m, bn_v)):
                nc.sync.dma_start(small[: src.shape[0], i : i + 1], src[:])
            nc.vector.memset(vs[:, 3:4], 1e-5)
            nc.scalar.activation(vs[:, 0:1], small[:, 4:5], ACTF.Sqrt,
                                 bias=vs[:, 3:4])
            nc.vector.reciprocal(vs[:, 1:2], vs[:, 0:1])
            nc.vector.tensor_mul(vs[:, 2:3], small[:, 1:2], vs[:, 1:2])
            nc.vector.tensor_mul(sh[:, 0:1], small[:, 3:4], vs[:, 2:3])
            nc.vector.tensor_tensor(sh[:, 1:2], small[:, 2:3], sh[:, 0:1],
                                    op=ALU.subtract)
            nc.vector.tensor_mul(sh[:, 0:1], small[:, 0:1], vs[:, 2:3])
            nc.vector.tensor_add(sh[:, 1:2], sh[:, 1:2], sh[:, 0:1])
            # dvp columns of feature + XT tail chunks
            for si in range(c.nbt):
                dv = pool.tile([P, c.dvp], F32, tag="dv")
                nc.scalar.dma_start(dv[:], dvp_nat[si * P : (si + 1) * P, :])
                nc.scalar.dma_start(feat_o[si * P : (si + 1) * P, 0 : c.dvp], dv[:])
            for k in range(kfeat - 2 * fq):
                kp = min(P, c.dvp - k * P)
                nc.gpsimd.dma_start(
                    XT_v[:kp, k, :], dvpT[k * P : k * P + kp, :])

        # ---------------- GCN epilogue (per graph, after its RS) ----------
        def epilogue(gi, hf):
            XT = PRE["XT"]
            gcn_b_sb = PRE["gcn_b_sb"]
            half = c.nbt // 2 if c.nbt >= 2 else c.nbt
            with tc.tile_pool(name=f"ep{gi}{hf}", bufs=4) as pool, \
                 tc.tile_pool(name=f"ep{gi}{hf}ps", bufs=4, space="PSUM") as psp:
                for sl in range(half):
                    s = hf * half + sl
                    tt = gi * c.nbt + s
                    if gi == 0:
                        po, ro = part_out_d, s * P
                    elif hf == 0:
                        po, ro = part_out_pa, sl * P
                    else:
                        po, ro = part_out_pb, sl * P
                    pr = pool.tile([P, c.f], F16, tag="pr")
                    nc.scalar.dma_start(pr[:], po[ro : ro + P, :])
                    a = pool.tile([P, c.f], F32, tag="a32")
                    nc.vector.tensor_scalar_mul(a[:], pr[:], dinv_b[:, tt : tt + 1])
                    nc.vector.tensor_add(a[:], a[:], gcn_b_sb[gi][:])
                    act = pool.tile([P, c.f], F32, tag="act")
                    nc.scalar.activation(act[:], a[:], ACTF.Lrelu, alpha=0.01)
                    col0 = c.dvp + gi * c.f
                    nc.scalar.dma_start(
                        feat_o[s * P : (s + 1) * P, col0 : col0 + c.f], act[:])
                    a16 = pool.tile([P, c.f], F16, tag="a16")
                    nc.vector.tensor_copy(a16[:], act[:])
                    for q in range(c.f // P):
                        pst = psp.tile([P, P], F16, tag="tp")
                        nc.tensor.transpose(
                            pst[:], a16[:, q * P : (q + 1) * P], ident_sb[:])
                        nc.vector.tensor_copy(
                            XT[:, gi * (c.f // P) + q, s * P : (s + 1) * P], pst[:])

        # ---------------- phase 2: batch-expanded partial aggregation ----
        def phase2(g_tab, idx, ewT, dsT, nch, part_in, gidx,
                   tile_order=None, snd_grp=None, part_row=None):
            if tile_order is None:
                tile_order = list(range(c.nbt_g))
            if snd_grp is None:
                snd_grp = c.nbt
            if part_row is None:
                part_row = lambda t: (part_in, t * P)
            ncols = c.nbt_g * nch
            with tc.tile_pool(name="p2c", bufs=1) as cp, \
                 tc.tile_pool(name="p2", bufs=4) as pool, \
                 tc.tile_pool(name="p2snd", bufs=3) as sndpool, \
                 tc.tile_pool(name="p2m", bufs=8) as mpool, \
                 tc.tile_pool(name="p2g", bufs=6) as gpool, \
                 tc.tile_pool(name="p2ps", bufs=4, space="PSUM") as psp:
                idx_sb = cp.tile([P, ncols * 8], I16, tag="idx")
                nc.sync.dma_start(idx_sb[:], idx[:])
                ew32 = cp.tile([P, ncols], F32, tag="ew32")
                nc.sync.dma_start(ew32[:], ewT[:])
                ds_sb = cp.tile([P, ncols], F32, tag="ds")
                nc.sync.dma_start(ds_sb[:], dsT[:])
                if gather_runtime:
                    for i in range(6):
                        gb0 = gpool.tile([P, nch, c.f], F16, tag="gb")
                        nc.vector.memset(gb0[:], 0.0)
                    nregs = [nc.gpsimd.alloc_register(f"nidx{gidx}_{i}")
                             for i in range(8)]
                sbuf_sends = []
                for ti, t in enumerate(tile_order):
                    gb = gpool.tile([P, nch, c.f], F16, tag="gb")
                    if gather_runtime:
                        nreg = nregs[t % 8]
                        nc.gpsimd.reg_load(nreg, cnt_sb[0:1, gidx * c.nbt_g + t :
                                                        gidx * c.nbt_g + t + 1])
                        nidx = nreg
                    else:
                        nidx = nch * P
                    nc.gpsimd.dma_gather(
                        gb[:], g_tab, idx_sb[:, t * nch * 8 : (t + 1) * nch * 8],
                        nch * P, nidx, c.f)
                    psa = psp.tile([P, fh], F32, tag="apsa")
                    psb = psp.tile([P, fh], F32, tag="apsb")
                    for ch in range(nch):
                        ci = t * nch + ch
                        mt = mpool.tile([P, P], F16, tag="mt")
                        nc.vector.tensor_scalar(
                            mt[:], iota_sb[:],
                            ds_sb[:, ci : ci + 1], ew32[:, ci : ci + 1],
                            op0=ALU.is_equal, op1=ALU.mult)
                        for nh, pstile in enumerate((psa, psb)):
                            nc.tensor.matmul(
                                pstile[:],
                                lhsT=mt[:], rhs=gb[:, ch, nh * fh : (nh + 1) * fh],
                                start=(ch == 0), stop=(ch == nch - 1))
                    if ti % snd_grp == 0:
                        snd = sndpool.tile([P, snd_grp, c.f], F16, tag="snd")
                        sbuf_sends.append(snd)
                    nc.vector.tensor_copy(snd[:, ti % snd_grp, :fh], psa[:])
                    nc.scalar.copy(snd[:, ti % snd_grp, fh:], psb[:])
                    if ti % snd_grp == snd_grp - 1:
                        buf, r0 = part_row(tile_order[ti - snd_grp + 1])
                        nc.sync.dma_start(
                            buf[r0 : r0 + snd_grp * P, :].rearrange(
                                "(s p) f -> p s f", p=P), snd[:])

        if 1 in phases:
            Wp_sb = load_w(Wp, kch_p, "wp")
            dinv_p = calc_dinv(bucket_p, c.np_pad // P, "p")
            phase1(xpT, Wp_sb, dinv_p, g_p, c.np_pad // P, kch_p)
        p1wr.release()
        if 2 in phases:
            phase2(g_d, idx_d, ewT_d, dsT_d, nch_d, part_in_d, 0)
            phase2(g_p, idx_p, ewT_p, dsT_p, nch_p, part_in_p, 1)
            if 25 in phases:
                nc.gpsimd.collective_compute(
                    "ReduceScatter", ALU.add,
                    replica_groups=[list(range(NCORES))],
                    ins=[part_in_d[:].opt()], outs=[part_out_d.opt()])
                if 3 in phases:
                    epilogue(0, 0)
                    if split_p:
                        epilogue(0, 1)
                nc.gpsimd.collective_compute(
                    "ReduceScatter", ALU.add,
                    replica_groups=[list(range(NCORES))],
                    ins=[part_in_pa[:].opt()], outs=[part_out_pa.opt()])
                if 3 in phases:
                    epilogue(1, 0)
                if split_p:
                    nc.gpsimd.collective_compute(
                        "ReduceScatter", ALU.add,
                        replica_groups=[list(range(NCORES))],
                        ins=[part_in_pb[:].opt()], outs=[part_out_pb.opt()])
                    if 3 in phases:
                        epilogue(1, 1)

        # ---------------- phase 3: epilogue + MLP -------------------------
        if 3 not in phases:
            const.release()
            tc.__exit__(None, None, None) if False else None
        kfeat = 2 * (c.f // P) + (c.dvp + P - 1) // P      # 27 XT chunks
        mh1 = c.enc_h // P
        m2 = c.enc_o // P

        encT_p = tc.alloc_tile_pool(name="encT", bufs=1)
        encT = encT_p.tile([P, m2, c.bsh], F16)
        d1_p = tc.alloc_tile_pool(name="d1o", bufs=1)
        dec1T = d1_p.tile([P, mh1, c.bsh], F16)
        xt_p = tc.alloc_tile_pool(name="xt", bufs=1)
        XT = xt_p.tile([P, kfeat, c.bsh], F16)

        with tc.tile_pool(name="p3", bufs=4) as pool, \
             tc.tile_pool(name="p3ps", bufs=4, space="PSUM") as psp:
            gcn_b_sb = {}
            for gi, bsrc in enumerate((gcn_bd, gcn_bp)):
                bt = const.tile([P, c.f], F32, tag=f"gb{gi}")
                nc.sync.dma_start(bt[:], bsrc[:])
                gcn_b_sb[gi] = bt
            for tt in range(2 * c.nbt):
                gi, s = tt // c.nbt, tt % c.nbt
                pr = pool.tile([P, c.f], F16, tag="pr")
                po = part_out_d if gi == 0 else part_out_p
                nc.sync.dma_start(pr[:], po[s * P : (s + 1) * P, :])
                a = pool.tile([P, c.f], F32, tag="a32")
                nc.vector.tensor_scalar_mul(a[:], pr[:], dinv_b[:, tt : tt + 1])
                nc.vector.tensor_add(a[:], a[:], gcn_b_sb[gi][:])
                act = pool.tile([P, c.f], F32, tag="act")
                nc.scalar.activation(act[:], a[:], ACTF.Lrelu, alpha=0.01)
                col0 = c.dvp + gi * c.f
                nc.sync.dma_start(
                    feat_o[s * P : (s + 1) * P, col0 : col0 + c.f], act[:])
                a16 = pool.tile([P, c.f], F16, tag="a16")
                nc.vector.tensor_copy(a16[:], act[:])
                for q in range(c.f // P):
                    pst = psp.tile([P, P], F16, tag="tp")
                    nc.tensor.transpose(
                        pst[:], a16[:, q * P : (q + 1) * P], ident_sb[:])
                    nc.vector.tensor_copy(
                        XT[:, gi * (c.f // P) + q, s * P : (s + 1) * P], pst[:])
            # dvp columns of feature (plain copy through SBUF) + XT chunks
            for s in range(c.nbt):
                dv = pool.tile([P, c.dvp], F32, tag="dv")
                nc.sync.dma_start(dv[:], dvp_nat[s * P : (s + 1) * P, :])
                nc.sync.dma_start(feat_o[s * P : (s + 1) * P, 0 : c.dvp], dv[:])
            for k in range(kfeat - 2 * fq):
                kp = min(P, c.dvp - k * P)
                nc.gpsimd.dma_start(
                    XT_v[:kp, k, :], dvpT[k * P : k * P + kp, :])

        # encoder layer 1: [feat -> enc_h], feature-major out1T
        l1_p = tc.alloc_tile_pool(name="l1o", bufs=1)
        out1T = l1_p.tile([P, mh1, c.bsh], F16)
        with tc.tile_pool(name="el1", bufs=4) as wpool, \
             tc.tile_pool(name="el1ps", bufs=1, space="PSUM") as psp:
            b1sb = const.tile([P, mh1], F32, tag="b1")
            nc.sync.dma_start(b1sb[:], b1t[:])
            ps1 = []
            for m in range(mh1):
                ps1_m = psp.tile([P, c.bsh], F32, tag=f"ps1_{m}")
                ps1.append(ps1_m)
            for k in range(kfeat):
                kp = min(P, c.feat - k * P)
                w16 = wpool.tile([P, c.enc_h], F16, tag="w1s16")
                nc.gpsimd.dma_start(w16[:kp, :], W1r[k * P : k * P + kp, :])
                for m in range(mh1):
                    nc.tensor.matmul(
                        ps1[m][:], lhsT=w16[:kp, m * P : (m + 1) * P],
                        rhs=XT[:kp, k, :], start=(k == 0), stop=(k == kfeat - 1))
            for m in range(mh1):
                nc.scalar.activation(out1T[:, m, :], ps1[m][:], ACTF.Lrelu,
                                     bias=b1sb[:, m : m + 1], alpha=0.01)

        # encoder layer 2 -> encT [P, enc_o/P, bsh]
        with tc.tile_pool(name="el2", bufs=2) as pool, \
             tc.tile_pool(name="el2ps", bufs=2 + m2, space="PSUM") as psp:
            W2sb = pool.tile([P, mh1, c.enc_o], F16, tag="W2")
            for k in range(mh1):
                w32 = pool.tile([P, c.enc_o], F32, tag="w2s")
                nc.sync.dma_start(w32[:], W2[k * P : (k + 1) * P, :])
                nc.vector.tensor_copy(W2sb[:, k, :], w32[:])
            b2sb = const.tile([P, m2], F32, tag="b2")
            nc.sync.dma_start(b2sb[:], b2t[:])
            for m in range(m2):
                ps = psp.tile([P, c.bsh], F32, tag="ps2")
                for k in range(mh1):
                    nc.tensor.matmul(
                        ps[:], lhsT=W2sb[:, k, m * P : (m + 1) * P],
                        rhs=out1T[:, k, :], start=(k == 0), stop=(k == mh1 - 1))
                nc.scalar.activation(encT[:, m, :], ps[:], ACTF.Lrelu,
                                     bias=b2sb[:, m : m + 1], alpha=0.01)
            # encoded output (batch-major) via PE transpose
            for m in range(m2):
                for s in range(c.nbt):
                    pst = psp.tile([P, P], F16, tag="etp")
                    nc.tensor.transpose(
                        pst[:], encT[:, m, s * P : (s + 1) * P], ident_sb[:])
                    ob = pool.tile([P, P], F32, tag="eob")
                    nc.vector.tensor_copy(ob[:], pst[:])
                    nc.sync.dma_start(
                        enc_o_t[s * P : (s + 1) * P, m * P : (m + 1) * P], ob[:])

        # decoder layer 1 -> dec1T [P, mh1, bsh]
        # (release XT + out1T space before the big decoder weights arrive)
        l1_p.release()
        xt_p.release()
        with tc.tile_pool(name="dl1", bufs=2) as pool, \
             tc.tile_pool(name="dl1ps", bufs=4, space="PSUM") as psp:
            D1sb = pool.tile([P, m2, c.enc_h], F16, tag="D1")
            for k in range(m2):
                w32 = pool.tile([P, c.enc_h], F32, tag="d1s")
                nc.sync.dma_start(w32[:], D1[k * P : (k + 1) * P, :])
                nc.vector.tensor_copy(D1sb[:, k, :], w32[:])
            d1sb = const.tile([P, mh1], F32, tag="d1b")
            nc.sync.dma_start(d1sb[:], d1t[:])
            for m in range(mh1):
                ps = psp.tile([P, c.bsh], F32, tag="psd1")
                for k in range(m2):
                    nc.tensor.matmul(
                        ps[:], lhsT=D1sb[:, k, m * P : (m + 1) * P],
                        rhs=encT[:, k, :], start=(k == 0), stop=(k == m2 - 1))
                nc.scalar.activation(dec1T[:, m, :], ps[:], ACTF.Lrelu,
                                     bias=d1sb[:, m : m + 1], alpha=0.01)

        # decoder layer 2 (batch-major out) + decoded output
        with tc.tile_pool(name="dl2w", bufs=1) as wres, \
             tc.tile_pool(name="dl2", bufs=2) as pool, \
             tc.tile_pool(name="dl2ps", bufs=3, space="PSUM") as psp:
            D2sb = wres.tile([P, mh1, c.feat], F16, tag="D2")
            for k in range(mh1):
                w32 = pool.tile([P, c.feat], F32, tag="d2s")
                nc.sync.dma_start(w32[:], D2[k * P : (k + 1) * P, :])
                nc.vector.tensor_copy(D2sb[:, k, :], w32[:])
            d2b = wres.tile([P, c.feat], F32, tag="d2b")
            nc.sync.dma_start(d2b[:], d2tile[:])
            nn = -(-c.feat // fh)
            for s in range(c.nbt):
                for n in range(nn):
                    n0 = n * fh
                    nw = min(fh, c.feat - n0)
                    ps = psp.tile([P, fh], F32, tag="psd2")
                    for k in range(mh1):
                        nc.tensor.matmul(
                            ps[:, :nw], lhsT=dec1T[:, k, s * P : (s + 1) * P],
                            rhs=D2sb[:, k, n0 : n0 + nw],
                            start=(k == 0), stop=(k == mh1 - 1))
                    ot = pool.tile([P, fh], F32, tag="dot")
                    nc.vector.tensor_add(ot[:, :nw], ps[:, :nw], d2b[:, n0 : n0 + nw])
                    nc.scalar.activation(ot[:, :nw], ot[:, :nw], ACTF.Lrelu,
                                         alpha=0.01)
                    nc.sync.dma_start(
                        dec_o_t[s * P : (s + 1) * P, n0 : n0 + nw], ot[:, :nw])

        # output head
        with tc.tile_pool(name="oh", bufs=2) as pool, \
             tc.tile_pool(name="ohps", bufs=2, space="PSUM") as psp:
            OW1sb = pool.tile([P, m2, c.out_h], F16, tag="OW1")
            for k in range(m2):
                w32 = pool.tile([P, c.out_h], F32, tag="ow1s")
                nc.sync.dma_start(w32[:], OW1[k * P : (k + 1) * P, :])
                nc.vector.tensor_copy(OW1sb[:, k, :], w32[:])
            small = const.tile([P, 8], F32, tag="bn")
            for i, src in enumerate((ob1, bn_g, bn_b, bn_m, bn_v)):
                nc.sync.dma_start(small[: src.shape[0], i : i + 1], src[:])
            # scale = gamma * rsqrt(var+eps); shift = beta - mean*scale + b1*scale
            vs = pool.tile([P, 4], F32, tag="vs")
            nc.vector.memset(vs[:, 3:4], 1e-5)
            nc.scalar.activation(vs[:, 0:1], small[:, 4:5], ACTF.Sqrt,
                                 bias=vs[:, 3:4])
            nc.vector.reciprocal(vs[:, 1:2], vs[:, 0:1])
            nc.vector.tensor_mul(vs[:, 2:3], small[:, 1:2], vs[:, 1:2])  # scale
            sh = pool.tile([P, 2], F32, tag="sh")
            nc.vector.tensor_mul(sh[:, 0:1], small[:, 3:4], vs[:, 2:3])  # mean*scale
            nc.vector.tensor_tensor(sh[:, 1:2], small[:, 2:3], sh[:, 0:1],
                                    op=ALU.subtract)                      # beta - m*s
            nc.vector.tensor_mul(sh[:, 0:1], small[:, 0:1], vs[:, 2:3])  # b1*scale
            nc.vector.tensor_add(sh[:, 1:2], sh[:, 1:2], sh[:, 0:1])     # shift
            psh = psp.tile([P, c.bsh], F32, tag="psh")
            for k in range(m2):
                nc.tensor.matmul(psh[:], lhsT=OW1sb[:, k, :], rhs=encT[:, k, :],
                                 start=(k == 0), stop=(k == m2 - 1))
            h2 = pool.tile([P, c.bsh], F16, tag="h2")
            nc.scalar.activation(h2[:], psh[:], ACTF.Lrelu,
                                 bias=sh[:, 1:2], scale=vs[:, 2:3], alpha=0.01)
            ow2 = pool.tile([P, 1], F32, tag="ow2s")
            nc.sync.dma_start(ow2[:], OW2[:])
            ow216 = pool.tile([P, 1], F16, tag="ow216")
            nc.vector.tensor_copy(ow216[:], ow2[:])
            ob2sb = pool.tile([1, 1], F32, tag="ob2")
            nc.sync.dma_start(ob2sb[:], ob2[:])
            psy = psp.tile([1, c.bsh], F32, tag="psy")
            nc.tensor.matmul(psy[:], lhsT=ow216[:], rhs=h2[:], start=True,
                             stop=True)
            ysb = pool.tile([1, c.bsh], F32, tag="ysb")
            nc.scalar.activation(ysb[:], psy[:], ACTF.Identity,
                                 bias=ob2sb[0:1, 0:1])
            nc.sync.dma_start(y_o.rearrange("b o -> o b"), ysb[0:1, :])

        for _pool in (xt_p if False else None,):
            pass
        d1_p.release(); encT_p.release(); const.release()

    nc.compile()


# ----------------------------------------------------------------------------
# entry point
# ----------------------------------------------------------------------------

def run(inputs, cfg):
    in_maps, meta = host_plan(inputs, cfg)
    nc = bacc.Bacc("TRN2", target_bir_lowering=False, debug=False,
                   num_devices=NCORES)
    build_kernel(nc, cfg, meta)
    res = run_bass_kernel_spmd(nc, in_maps, core_ids=list(range(NCORES)))
    outs = res.results
    y = np.concatenate([outs[i]["y_part"] for i in range(NCORES)], 0)
    enc = np.concatenate([outs[i]["encoded_part"] for i in range(NCORES)], 0)
    dec = np.concatenate([outs[i]["decoded_part"] for i in range(NCORES)], 0)
    feat = np.concatenate([outs[i]["feature_part"] for i in range(NCORES)], 0)
    return (y, enc, dec, feat)


def kernel(**inputs):
    return run(inputs, Cfg())
